# revision 23
# baseline (speedup 1.0000x reference)
"""Trainium2 Bass kernel for nn_ActorNet (2-layer LSTM + BatchNorm + Gumbel sampling).

Strategy (v3):
- Data-parallel over batch: B=4096 -> 512 per core across 8 cores.
- Recurrent state TRANSPOSED in SBUF: [H on partitions, batch on free],
  wide as [128, 4*512]; h0/h1 double-buffered so cell updates overlap the
  next tiles' matmuls (no write-after-read serialization on h).
- Input path folded on host into rank-4 blocks; the 4 rank-4 matmuls of each
  batch-block are packed into the PE's four 32-row strips via tile_position
  and run concurrently (~1 matmul slot instead of 4).
- BatchNorm only affects the output head, not the recurrence. The T=256 steps
  are processed in pairs of 16-step chunks inside one hardware loop; each
  chunk's batch-stat partial sums are AllReduce'd per chunk *inside* the loop,
  and the BN + head + sampling work for pair i-1 runs interleaved with the
  LSTM matmuls of pair i (PE-bound), hiding nearly all of it. A static
  epilogue handles the last pair.
"""
import sys

if "/opt/trn_rl_repo" not in sys.path:
    sys.path.insert(0, "/opt/trn_rl_repo")

import contextlib

import numpy as np

import concourse.bass as bass
import concourse.tile as tile
from concourse import bacc, mybir
from concourse.bass_utils import run_bass_kernel_spmd

F32 = mybir.dt.float32
F16 = mybir.dt.float16
AF = mybir.ActivationFunctionType
ALU = mybir.AluOpType
AX = mybir.AxisListType

N_CORES = 8
B_GLOBAL = 4096
B = B_GLOBAL // N_CORES  # 512
H = 512
G4H = 4 * H              # 2048
O_SYM = 64
O_POS = 3
O_CAT = O_SYM + O_POS    # 67
BN_EPS = 1e-5

KT = H // 128            # 4 k-tiles per H
JT = G4H // 128          # 16 j-tiles over gate rows
NB = B // 128            # 4 batch tiles per core
U = 16                   # stats chunk (steps per collective)
P = 2 * U                # steps per loop body (pair of chunks)


def build(T: int, mode: str = "full"):
    # mode: "a" = phase A loop only (no collectives, no B);
    #       "b" = A + in-loop collectives (no B); "full" = everything
    assert T % P == 0
    NIT = T // P          # hardware-loop iterations (8)
    nc = bacc.Bacc("TRN2", target_bir_lowering=False, debug=False,
                   num_devices=N_CORES)

    def din(name, shape, dt=F32):
        return nc.dram_tensor(name, list(shape), dt, kind="ExternalInput").ap()

    w0_d = din("w0", (128, KT * G4H), F16)     # W_hh0T blocks
    w1i_d = din("w1i", (128, KT * G4H), F16)   # W_ih1T blocks (input h0)
    w1h_d = din("w1h", (128, KT * G4H), F16)   # W_hh1T blocks (input h1)
    dgt_d = din("dgt", (128, JT * 128), F16)   # row-tiled daug blocks
    b1v_d = din("b1v", (128, JT))              # b1 per j-tile
    wcat_d = din("wcat", (128, KT * O_CAT), F16)  # [W_sym; W_pos].T blocks
    bcat_d = din("bcat", (128, NB * O_CAT))    # head bias per partition
    gamw_d = din("gamw", (128, KT))
    betw_d = din("betw", (128, KT))
    hc0_d = din("hc0", (128, 4 * G4H))         # h0,c0,h1,c1 wide
    tok_d = din("tok", (T, 4, B), F16)         # [ones; tok0; tok1; tok2]
    gum_d = din("gum", (T, 128, NB * O_CAT))   # gumbel, sampling layout
    out_d = nc.dram_tensor("out", [T, 128, 2 * NB], F32,
                           kind="ExternalOutput").ap()

    # DRAM internals
    hist = nc.dram_tensor("h1_hist", [T, 128, G4H], F16).ap()
    NCH = 2 * (T // P)    # one stats buffer pair per 16-step chunk
    ccin = [nc.dram_tensor(f"cc_in{x}", [2, 128, U * KT], F32).ap()
            for x in range(NCH)]
    ccout = [nc.dram_tensor(f"cc_out{x}", [2, 128, U * KT], F32,
                            addr_space="Shared").ap() for x in range(NCH)]

    with tile.TileContext(nc) as tc:
        ctx = contextlib.ExitStack()
        with ctx:
            pc = ctx.enter_context(tc.tile_pool(name="const", bufs=1))
            pst = ctx.enter_context(tc.tile_pool(name="state", bufs=1))

            # ---------- load weights (already fp16 from host) ----------
            # split into k-blocks so the first step's matmuls start as soon
            # as the blocks they touch have landed
            w0 = pc.tile([128, KT * G4H], F16)
            for k in range(KT):
                nc.sync.dma_start(w0[:, k * G4H:(k + 1) * G4H],
                                  w0_d[:, k * G4H:(k + 1) * G4H])
            dgt = pc.tile([128, JT * 128], F16)
            nc.sync.dma_start(dgt[:], dgt_d[:])
            w1h = pc.tile([128, KT * G4H], F16)
            for k in range(KT):
                nc.sync.dma_start(w1h[:, k * G4H:(k + 1) * G4H],
                                  w1h_d[:, k * G4H:(k + 1) * G4H])
            w1i = pc.tile([128, KT * G4H], F16)
            for k in range(KT):
                nc.sync.dma_start(w1i[:, k * G4H:(k + 1) * G4H],
                                  w1i_d[:, k * G4H:(k + 1) * G4H])
            wcat = pc.tile([128, KT * O_CAT], F16)
            nc.sync.dma_start(wcat[:], wcat_d[:])
            b1v = pc.tile([128, JT], F32)
            nc.sync.dma_start(b1v[:], b1v_d[:])
            bcat = pc.tile([128, NB * O_CAT], F32)
            nc.sync.dma_start(bcat[:], bcat_d[:])
            gamw = pc.tile([128, KT], F32)
            nc.sync.dma_start(gamw[:], gamw_d[:])
            betw = pc.tile([128, KT], F32)
            nc.sync.dma_start(betw[:], betw_d[:])
            epsb = pc.tile([128, 1], F32)
            nc.gpsimd.memset(epsb[:], BN_EPS)

            # ---------- states: h double-buffered, c in-place ----------
            hbuf = [[pst.tile([128, G4H], F16, name=f"h{l}_{s}")
                     for s in range(2)] for l in range(2)]
            c0 = pst.tile([128, G4H], F32, name="c0")
            c1 = pst.tile([128, G4H], F32, name="c1")
            with tc.tile_pool(name="istage", bufs=1) as pi:
                ist = pi.tile([128, 4 * G4H], F32)
                nc.sync.dma_start(ist[:], hc0_d[:])
                nc.vector.tensor_copy(hbuf[0][0][:], ist[:, 0 * G4H:1 * G4H])
                nc.vector.tensor_copy(c0[:], ist[:, 1 * G4H:2 * G4H])
                nc.vector.tensor_copy(hbuf[1][0][:], ist[:, 2 * G4H:3 * G4H])
                nc.vector.tensor_copy(c1[:], ist[:, 3 * G4H:4 * G4H])

            pa = ctx.enter_context(tc.tile_pool(name="workA", bufs=1))
            ppa = ctx.enter_context(
                tc.tile_pool(name="psumA", bufs=7, space="PSUM"))
            pb = ctx.enter_context(tc.tile_pool(name="workB", bufs=1))
            ppb = ctx.enter_context(
                tc.tile_pool(name="psumB", bufs=2, space="PSUM"))
            psc = ctx.enter_context(tc.tile_pool(name="scsh", bufs=2))
            pbn = ctx.enter_context(tc.tile_pool(name="bn", bufs=2))

            # =================== phase A single step ===================
            def step_A(t, u, statsX):
                par = u % 2
                ul = u % U   # stats slot within the chunk
                h0o, h0n = hbuf[0][par], hbuf[0][1 - par]
                h1o, h1n = hbuf[1][par], hbuf[1][1 - par]
                tokr = pa.tile([128, B], F16, tag="tokr", bufs=3)
                for sb in range(4):
                    nc.sync.dma_start(tokr[32 * sb:32 * sb + 4, :], tok_d[t])

                # ----- layer 0: gate matmuls (jb-major) + packed daug -----
                gates = [pa.tile([128, G4H], F16, tag=f"gate{q}", bufs=2,
                                 name=f"g{q}") for q in range(4)]
                pss = {}
                for jb in range(NB):
                    for q in range(4):
                        j = q * NB + jb
                        ps = ppa.tile([128, 512], F32, tag="ps",
                                      name=f"ps0_{jb}_{q}")
                        pss[q] = ps
                        for k in range(KT):
                            nc.tensor.matmul(
                                ps[:],
                                w0[:, k * G4H + j * 128:k * G4H + (j + 1) * 128],
                                h0o[:, k * 512:(k + 1) * 512],
                                start=(k == 0), stop=False)
                    # 4 rank-4 input matmuls packed into the 4 row strips
                    for q in range(4):
                        j = q * NB + jb
                        nc.tensor.matmul(
                            pss[q][:],
                            dgt[32 * q:32 * q + 4, j * 128:(j + 1) * 128],
                            tokr[32 * q:32 * q + 4, :],
                            start=False, stop=True, tile_position=(32 * q, 0))
                    blk = slice(jb * 512, (jb + 1) * 512)
                    for q in range(4):
                        func = AF.Tanh if q == 2 else AF.Sigmoid
                        nc.scalar.activation(gates[q][:, blk], pss[q][:], func)
                    # cell update for this block (h0 double-buffered)
                    t1 = pa.tile([128, 512], F32, tag="t1", bufs=2,
                                 name=f"t1_0_{jb}")
                    nc.vector.tensor_mul(t1[:], gates[1][:, blk], c0[:, blk])
                    t2 = pa.tile([128, 512], F16, tag="t2", bufs=2,
                                 name=f"t2_0_{jb}")
                    nc.vector.tensor_mul(t2[:], gates[0][:, blk], gates[2][:, blk])
                    nc.vector.tensor_add(c0[:, blk], t1[:], t2[:])
                    tnc = pa.tile([128, 512], F16, tag="tnc", bufs=2,
                                  name=f"tnc_0_{jb}")
                    nc.scalar.activation(tnc[:], c0[:, blk], AF.Tanh)
                    nc.vector.tensor_mul(h0n[:, blk], gates[3][:, blk], tnc[:])

                # ----- layer 1 (jb-major; h1 part staggered one tile ahead of
                # the h0-new part so the first h0 reads never outrun the cell) -----
                gates1 = [pa.tile([128, G4H], F16, tag=f"gate{q}", bufs=2,
                                  name=f"g1{q}") for q in range(4)]
                ps1 = {}

                def l1_hpart(jb, q):
                    j = q * NB + jb
                    ps = ppa.tile([128, 512], F32, tag="ps",
                                  name=f"ps1_{jb}_{q}")
                    ps1[(jb, q)] = ps
                    for k in range(KT):
                        nc.tensor.matmul(
                            ps[:],
                            w1h[:, k * G4H + j * 128:k * G4H + (j + 1) * 128],
                            h1o[:, k * 512:(k + 1) * 512],
                            start=(k == 0), stop=False)

                def l1_ipart(jb, q):
                    j = q * NB + jb
                    ps = ps1.pop((jb, q))
                    for k in range(KT):
                        nc.tensor.matmul(
                            ps[:],
                            w1i[:, k * G4H + j * 128:k * G4H + (j + 1) * 128],
                            h0n[:, k * 512:(k + 1) * 512],
                            start=False, stop=(k == KT - 1))
                    blk = slice(jb * 512, (jb + 1) * 512)
                    func = AF.Tanh if q == 2 else AF.Sigmoid
                    nc.scalar.activation(gates1[q][:, blk], ps[:], func,
                                         bias=b1v[:, j:j + 1])

                def l1_cell(jb):
                    blk = slice(jb * 512, (jb + 1) * 512)
                    t1 = pa.tile([128, 512], F32, tag="t1", bufs=2,
                                 name=f"t1_1_{jb}")
                    nc.vector.tensor_mul(t1[:], gates1[1][:, blk], c1[:, blk])
                    t2 = pa.tile([128, 512], F16, tag="t2", bufs=2,
                                 name=f"t2_1_{jb}")
                    nc.vector.tensor_mul(t2[:], gates1[0][:, blk],
                                         gates1[2][:, blk])
                    nc.vector.tensor_add(c1[:, blk], t1[:], t2[:])
                    tnc = pa.tile([128, 512], F16, tag="tnc", bufs=2,
                                  name=f"tnc_1_{jb}")
                    nc.scalar.activation(tnc[:], c1[:, blk], AF.Tanh)
                    nc.vector.tensor_mul(h1n[:, blk], gates1[3][:, blk], tnc[:])
                    # batch stats (sum, sum of squares) for this block
                    sqs = pa.tile([128, 512], F32, tag="sqs", bufs=2,
                                  name=f"sqs_{jb}")
                    nc.scalar.activation(
                        sqs[:], h1n[:, blk], AF.Identity,
                        accum_out=statsX[:, ul * KT + jb:ul * KT + jb + 1])
                    sqs2 = pa.tile([128, 512], F32, tag="sqs2", bufs=2,
                                   name=f"sqs2_{jb}")
                    nc.scalar.activation(
                        sqs2[:], h1n[:, blk], AF.Square,
                        accum_out=statsX[:, 4 * U + ul * KT + jb:
                                         4 * U + ul * KT + jb + 1])

                seq = [(jb, q) for jb in range(NB) for q in range(4)]
                STG = 2  # h1-part tiles staged ahead of the h0-part
                for si in range(STG):
                    l1_hpart(*seq[si])
                for si in range(JT):
                    if si + STG < JT:
                        l1_hpart(*seq[si + STG])
                    l1_ipart(*seq[si])
                    if seq[si][1] == 3:
                        l1_cell(seq[si][0])
                nc.sync.dma_start(hist[t], h1n[:])

            # =================== BN scale/shift for one chunk ===================
            def bn_chunk(scale, shift, col0, ccoX):
                W = U * KT  # 64 cols
                cs = slice(col0, col0 + W)
                g1 = pbn.tile([128, W], F32, tag="g1")
                nc.sync.dma_start(g1[:], ccoX[0])
                g2 = pbn.tile([128, W], F32, tag="g2")
                nc.sync.dma_start(g2[:], ccoX[1])
                mean = pbn.tile([128, W], F32, tag="mean")
                nc.vector.tensor_scalar(mean[:], g1[:], 1.0 / B_GLOBAL,
                                        None, op0=ALU.mult)
                var = pbn.tile([128, W], F32, tag="var")
                nc.vector.tensor_scalar(var[:], g2[:], 1.0 / B_GLOBAL,
                                        None, op0=ALU.mult)
                msq = pbn.tile([128, W], F32, tag="msq")
                nc.vector.tensor_mul(msq[:], mean[:], mean[:])
                nc.vector.tensor_sub(var[:], var[:], msq[:])
                lnv = pbn.tile([128, W], F32, tag="lnv")
                nc.scalar.activation(lnv[:], var[:], AF.Ln, bias=epsb[:])
                rstd = pbn.tile([128, W], F32, tag="rstd")
                nc.scalar.activation(rstd[:], lnv[:], AF.Exp, scale=-0.5)
                gam_bc = gamw[:].unsqueeze(1).broadcast_to([128, U, KT])
                bet_bc = betw[:].unsqueeze(1).broadcast_to([128, U, KT])
                sc3 = scale[:, cs].rearrange("p (u k) -> p u k", k=KT)
                sh3 = shift[:, cs].rearrange("p (u k) -> p u k", k=KT)
                nc.vector.tensor_tensor(
                    sc3, rstd[:].rearrange("p (u k) -> p u k", k=KT),
                    gam_bc, op=ALU.mult)
                nc.vector.tensor_mul(msq[:], mean[:], scale[:, cs])
                nc.vector.tensor_tensor(
                    sh3, bet_bc,
                    msq[:].rearrange("p (u k) -> p u k", k=KT),
                    op=ALU.subtract)

            # =================== phase B single step ===================
            def step_B(vt, ub, scale, shift, s_acc, out_acc):
                h1t = pb.tile([128, G4H], F16, tag="h1t", bufs=3)
                nc.sync.dma_start(h1t[:], hist[vt])
                gum = pb.tile([128, NB * O_CAT], F32, tag="gum", bufs=3)
                nc.sync.dma_start(gum[:], gum_d[vt])

                ysq = pb.tile([128, G4H], F16, tag="ysq", bufs=2)
                for k in range(KT):
                    c = ub * KT + k
                    nc.scalar.activation(
                        ysq[:, k * 512:(k + 1) * 512],
                        h1t[:, k * 512:(k + 1) * 512], AF.Square,
                        bias=shift[:, c:c + 1], scale=scale[:, c:c + 1])
                gaus = pb.tile([128, G4H], F16, tag="gaus", bufs=2)
                nc.scalar.activation(gaus[:], ysq[:], AF.Exp, scale=-1.0)

                ps = ppb.tile([128, NB * O_CAT], F32, tag="psb", bufs=1)
                for bb in range(NB):
                    sl = slice(bb * O_CAT, (bb + 1) * O_CAT)
                    for k in range(KT):
                        nc.tensor.matmul(
                            ps[:, sl],
                            gaus[:, k * 512 + bb * 128:k * 512 + (bb + 1) * 128],
                            wcat[:, k * O_CAT:(k + 1) * O_CAT],
                            start=(k == 0), stop=(k == KT - 1))
                z = pb.tile([128, NB * O_CAT], F32, tag="z", bufs=2)
                nc.vector.tensor_add(z[:], ps[:], bcat[:])

                z3 = z[:].rearrange("p (b o) -> p b o", b=NB)
                # softmax denominators without max-subtraction (|z| small)
                ez = pb.tile([128, NB * O_CAT], F32, tag="ez", bufs=2)
                nc.scalar.activation(ez[:], z[:], AF.Exp)
                ez3 = ez[:].rearrange("p (b o) -> p b o", b=NB)
                s8 = s_acc[:, ub * 8:(ub + 1) * 8]
                nc.vector.tensor_reduce(s8[:, 0:NB], ez3[:, :, 0:O_SYM],
                                        axis=AX.X, op=ALU.add)
                nc.vector.tensor_reduce(s8[:, NB:], ez3[:, :, O_SYM:O_CAT],
                                        axis=AX.X, op=ALU.add)
                # gumbel-max: lp_sel = (z+gum)_max - gum_sel - ln(s)
                tg = pb.tile([128, NB * O_CAT], F32, tag="tg", bufs=2)
                nc.vector.tensor_add(tg[:], z[:], gum[:])
                tg3 = tg[:].rearrange("p (b o) -> p b o", b=NB)
                t8 = pb.tile([128, 2 * NB], F32, tag="t8", bufs=2)
                nc.vector.tensor_reduce(t8[:, 0:NB], tg3[:, :, 0:O_SYM],
                                        axis=AX.X, op=ALU.max)
                nc.vector.tensor_reduce(t8[:, NB:], tg3[:, :, O_SYM:O_CAT],
                                        axis=AX.X, op=ALU.max)
                mask = pb.tile([128, NB * O_CAT], F32, tag="mask", bufs=2)
                mask3 = mask[:].rearrange("p (b o) -> p b o", b=NB)
                nc.vector.tensor_tensor(
                    mask3[:, :, 0:O_SYM], tg3[:, :, 0:O_SYM],
                    t8[:, 0:NB].unsqueeze(2).broadcast_to([128, NB, O_SYM]),
                    op=ALU.is_equal)
                nc.vector.tensor_tensor(
                    mask3[:, :, O_SYM:O_CAT], tg3[:, :, O_SYM:O_CAT],
                    t8[:, NB:].unsqueeze(2).broadcast_to([128, NB, O_POS]),
                    op=ALU.is_equal)
                gsel = pb.tile([128, NB * O_CAT], F32, tag="gsel", bufs=2)
                nc.vector.tensor_mul(gsel[:], gum[:], mask[:])
                gsel3 = gsel[:].rearrange("p (b o) -> p b o", b=NB)
                g8 = pb.tile([128, 2 * NB], F32, tag="g8", bufs=2)
                nc.vector.tensor_reduce(g8[:, 0:NB], gsel3[:, :, 0:O_SYM],
                                        axis=AX.X, op=ALU.add)
                nc.vector.tensor_reduce(g8[:, NB:], gsel3[:, :, O_SYM:O_CAT],
                                        axis=AX.X, op=ALU.add)
                nc.vector.tensor_sub(out_acc[:, ub * 8:(ub + 1) * 8],
                                     t8[:], g8[:])

            def pair_B(vt0, scale, shift):
                """One pair (P steps) of phase B work; vt0 = loop-var expr."""
                s_acc = pb.tile([128, P * 8], F32, tag="s_acc", bufs=2)
                out_acc = pb.tile([128, P * 8], F32, tag="out_acc", bufs=2)
                for ub in range(P):
                    step_B(vt0 + ub, ub, scale, shift, s_acc, out_acc)
                ln_acc = pb.tile([128, P * 8], F32, tag="ln_acc", bufs=2)
                nc.scalar.activation(ln_acc[:], s_acc[:], AF.Ln)
                nc.vector.tensor_sub(out_acc[:], out_acc[:], ln_acc[:])
                return out_acc

            def emit_pair_out(out_acc, tv):
                nc.sync.dma_start(
                    out_d[bass.ts(tv, P)].transpose([1, 0, 2]),
                    out_acc[:].rearrange("p (t c) -> p t c", c=8))

            RG = [list(range(N_CORES))]

            # ========== main sweep: unrolled pairs; B rides one pair behind ==========
            scaleE = shiftE = sE_acc = oE_acc = None
            for tv in range(NIT):
                last = tv == NIT - 1
                statsA = pa.tile([128, 8 * U], F32, tag="statsA", bufs=2,
                                 name=f"stA{tv}")
                statsB = pa.tile([128, 8 * U], F32, tag="statsB", bufs=2,
                                 name=f"stB{tv}")
                do_B = mode == "full" and tv >= 1
                if do_B:
                    scale = psc.tile([128, P * KT], F32, tag="scale",
                                     name=f"sc{tv}")
                    shift = psc.tile([128, P * KT], F32, tag="shift",
                                     name=f"sh{tv}")
                    s_acc = pb.tile([128, P * 8], F32, tag="s_acc",
                                    bufs=2, name=f"sa{tv}")
                    out_acc = pb.tile([128, P * 8], F32, tag="out_acc",
                                      bufs=2, name=f"oa{tv}")
                    vb0 = (tv - 1) * P  # first step of the pair B processes

                # first two A steps lead; BN math for the previous pair
                step_A(tv * P + 0, 0, statsA)
                step_A(tv * P + 1, 1, statsA)
                if do_B:
                    bn_chunk(scale, shift, 0, ccout[2 * tv - 2])
                    bn_chunk(scale, shift, U * KT, ccout[2 * tv - 1])
                for u in range(2, P):
                    step_A(tv * P + u, u, statsA if u < U else statsB)
                    if do_B:
                        step_B(vb0 + (u - 2), u - 2, scale, shift,
                               s_acc, out_acc)
                    if u == U - 1:
                        nc.sync.dma_start(ccin[2 * tv][0], statsA[:, 0:4 * U])
                        nc.sync.dma_start(ccin[2 * tv][1],
                                          statsA[:, 4 * U:8 * U])
                        if mode != "a":
                            nc.gpsimd.collective_compute(
                                "AllReduce", ALU.add, replica_groups=RG,
                                ins=[ccin[2 * tv].opt()],
                                outs=[ccout[2 * tv].opt()])
                    if mode == "full" and last and u == U + 1:
                        # start pulling the final pair's first chunk of B
                        # work into this iteration (its stats just reduced)
                        scaleE = psc.tile([128, P * KT], F32, tag="scale",
                                          name="scE")
                        shiftE = psc.tile([128, P * KT], F32, tag="shift",
                                          name="shE")
                        sE_acc = pb.tile([128, P * 8], F32, tag="s_acc",
                                         bufs=2, name="saE")
                        oE_acc = pb.tile([128, P * 8], F32, tag="out_acc",
                                         bufs=2, name="oaE")
                        bn_chunk(scaleE, shiftE, 0, ccout[2 * tv])
                    if mode == "full" and last and u >= U + 2:
                        ub2 = u - (U + 2)
                        step_B(tv * P + ub2, ub2, scaleE, shiftE,
                               sE_acc, oE_acc)
                if do_B:
                    step_B(vb0 + (P - 2), P - 2, scale, shift, s_acc, out_acc)
                    step_B(vb0 + (P - 1), P - 1, scale, shift, s_acc, out_acc)
                nc.sync.dma_start(ccin[2 * tv + 1][0], statsB[:, 0:4 * U])
                nc.sync.dma_start(ccin[2 * tv + 1][1], statsB[:, 4 * U:8 * U])
                if mode != "a":
                    nc.gpsimd.collective_compute(
                        "AllReduce", ALU.add, replica_groups=RG,
                        ins=[ccin[2 * tv + 1].opt()],
                        outs=[ccout[2 * tv + 1].opt()])
                if do_B:
                    ln_acc = pb.tile([128, P * 8], F32, tag="ln_acc", bufs=2,
                                     name=f"ln{tv}")
                    nc.scalar.activation(ln_acc[:], s_acc[:], AF.Ln)
                    nc.vector.tensor_sub(out_acc[:], out_acc[:], ln_acc[:])
                    emit_pair_out(out_acc, tv - 1)

            # ============ epilogue: finish the final pair of phase B ============
            if mode == "full":
                bn_chunk(scaleE, shiftE, U * KT, ccout[NCH - 1])
                for ub in range(P - U - 2, P):
                    step_B(T - P + ub, ub, scaleE, shiftE, sE_acc, oE_acc)
                ln_acc = pb.tile([128, P * 8], F32, tag="ln_acc", bufs=2,
                                 name="lnE")
                nc.scalar.activation(ln_acc[:], sE_acc[:], AF.Ln)
                nc.vector.tensor_sub(oE_acc[:], oE_acc[:], ln_acc[:])
                nc.sync.dma_start(
                    out_d[T - P:T].transpose([1, 0, 2]),
                    oE_acc[:].rearrange("p (t c) -> p t c", c=8))

    nc.compile()
    return nc


def prep_inputs(emb, W_in, b_in, W_ih0, W_hh0, b0, W_ih1, W_hh1, b1,
                gamma, beta, W_sym, b_sym, W_pos, b_pos,
                h_init, c_init, tokens, gumbel_sym, gumbel_pos, T):
    """Host-side preprocessing -> per-core input maps."""
    f64 = np.float64

    def wide(mat_t, dt=np.float16):  # [H, N] -> [128, KT*N]
        Hh, N = mat_t.shape
        return np.ascontiguousarray(
            mat_t.reshape(Hh // 128, 128, N).transpose(1, 0, 2).reshape(128, -1)
        ).astype(dt)

    w0_h = wide(W_hh0.T)
    w1i_h = wide(W_ih1.T)
    w1h_h = wide(W_hh1.T)

    Wc = W_ih0.astype(f64) @ W_in.astype(f64)            # [2048, 24]
    embd = emb.astype(f64)
    base = np.tile(embd[0], 3)                           # [24]
    delta = embd[1] - embd[0]                            # [8]
    c0v = Wc @ base + b0.astype(f64) + b_in.astype(f64) @ W_ih0.T.astype(f64)
    dvecs = [Wc[:, 8 * j:8 * (j + 1)] @ delta for j in range(3)]
    daug_h = np.stack([c0v] + dvecs).astype(np.float32)  # [4, 2048]
    # row-tiled daug: block for j-tile j=q*NB+jb lives at partitions 32q..32q+4
    dgt_h = np.zeros((128, JT * 128), np.float16)
    for j in range(JT):
        q = j // NB
        dgt_h[32 * q:32 * q + 4, j * 128:(j + 1) * 128] = \
            daug_h[:, j * 128:(j + 1) * 128]

    b1v_h = np.ascontiguousarray(b1.reshape(JT, 128).T).astype(np.float32)
    Wcat = np.concatenate([W_sym, W_pos], axis=0)        # [67, 512]
    wcat_h = wide(Wcat.T)
    bcat_h = np.tile(np.concatenate([b_sym, b_pos])[None, :],
                     (128, NB)).astype(np.float32)
    gamw_h = np.ascontiguousarray(gamma.reshape(KT, 128).T).astype(np.float32)
    betw_h = np.ascontiguousarray(beta.reshape(KT, 128).T).astype(np.float32)

    in_maps = []
    for c in range(N_CORES):
        bs = slice(c * B, (c + 1) * B)
        hc = np.concatenate([
            wide(h_init[0, bs].T, np.float32), wide(c_init[0, bs].T, np.float32),
            wide(h_init[1, bs].T, np.float32), wide(c_init[1, bs].T, np.float32)],
            axis=1)
        tok_h = np.empty((T, 4, B), np.float16)
        tok_h[:, 0, :] = 1.0
        tok_h[:, 1:4, :] = tokens[:, bs, :].transpose(0, 2, 1).astype(np.float16)
        gcat = np.concatenate(
            [gumbel_sym[:, bs, :], gumbel_pos[:, bs, :]], axis=2
        ).astype(np.float32)
        gum_h = np.ascontiguousarray(
            gcat.reshape(T, NB, 128, O_CAT).transpose(0, 2, 1, 3)
            .reshape(T, 128, NB * O_CAT))
        in_maps.append({
            "w0": w0_h, "w1i": w1i_h, "w1h": w1h_h, "dgt": dgt_h,
            "b1v": b1v_h, "wcat": wcat_h, "bcat": bcat_h,
            "gamw": gamw_h, "betw": betw_h,
            "hc0": np.ascontiguousarray(hc),
            "tok": tok_h, "gum": gum_h,
        })
    return in_maps


_NC_CACHE = {}


def run(inputs: dict, T: int, trace: bool = False):
    if T not in _NC_CACHE:
        _NC_CACHE[T] = build(T)
    nc = _NC_CACHE[T]
    in_maps = prep_inputs(T=T, **inputs)
    try:
        res = run_bass_kernel_spmd(nc, in_maps, core_ids=list(range(N_CORES)),
                                   trace=trace)
    except Exception:
        # a previous crash can leave the device wedged; reset and retry once
        try:
            import ctypes
            ctypes.CDLL("/opt/axon/libaxon_pjrt.so").axon_reset()
        except Exception:
            pass
        res = run_bass_kernel_spmd(nc, in_maps, core_ids=list(range(N_CORES)),
                                   trace=trace)
    # per-core staging [T, 128, 2*NB] -> [2, T, 512]
    outs = [r["out"].reshape(T, 128, 2, NB).transpose(2, 0, 3, 1)
            .reshape(2, T, B) for r in res.results]
    out = np.concatenate(outs, axis=2)
    return out, res


def kernel(**inputs) -> np.ndarray:
    inputs = {k: np.asarray(v) for k, v in inputs.items()}
    T = inputs["tokens"].shape[0]
    out, _ = run(inputs, T)
    return out.astype(np.float32)


# revision 24
# speedup vs baseline: 1.0041x; 1.0041x over previous
"""Trainium2 Bass kernel for nn_ActorNet (2-layer LSTM + BatchNorm + Gumbel sampling).

Strategy (v3):
- Data-parallel over batch: B=4096 -> 512 per core across 8 cores.
- Recurrent state TRANSPOSED in SBUF: [H on partitions, batch on free],
  wide as [128, 4*512]; h0/h1 double-buffered so cell updates overlap the
  next tiles' matmuls (no write-after-read serialization on h).
- Input path folded on host into rank-4 blocks; the 4 rank-4 matmuls of each
  batch-block are packed into the PE's four 32-row strips via tile_position
  and run concurrently (~1 matmul slot instead of 4).
- BatchNorm only affects the output head, not the recurrence. The T=256 steps
  are processed in pairs of 16-step chunks inside one hardware loop; each
  chunk's batch-stat partial sums are AllReduce'd per chunk *inside* the loop,
  and the BN + head + sampling work for pair i-1 runs interleaved with the
  LSTM matmuls of pair i (PE-bound), hiding nearly all of it. A static
  epilogue handles the last pair.
"""
import sys

if "/opt/trn_rl_repo" not in sys.path:
    sys.path.insert(0, "/opt/trn_rl_repo")

import contextlib

import numpy as np

import concourse.bass as bass
import concourse.tile as tile
from concourse import bacc, mybir
from concourse.bass_utils import run_bass_kernel_spmd

F32 = mybir.dt.float32
F16 = mybir.dt.float16
AF = mybir.ActivationFunctionType
ALU = mybir.AluOpType
AX = mybir.AxisListType

N_CORES = 8
B_GLOBAL = 4096
B = B_GLOBAL // N_CORES  # 512
H = 512
G4H = 4 * H              # 2048
O_SYM = 64
O_POS = 3
O_CAT = O_SYM + O_POS    # 67
BN_EPS = 1e-5

KT = H // 128            # 4 k-tiles per H
JT = G4H // 128          # 16 j-tiles over gate rows
NB = B // 128            # 4 batch tiles per core
U = 16                   # stats chunk (steps per collective)
P = 2 * U                # steps per loop body (pair of chunks)


def build(T: int, mode: str = "full"):
    # mode: "a" = phase A loop only (no collectives, no B);
    #       "b" = A + in-loop collectives (no B); "full" = everything
    assert T % P == 0
    NIT = T // P          # hardware-loop iterations (8)
    nc = bacc.Bacc("TRN2", target_bir_lowering=False, debug=False,
                   num_devices=N_CORES)

    def din(name, shape, dt=F32):
        return nc.dram_tensor(name, list(shape), dt, kind="ExternalInput").ap()

    w0_d = din("w0", (128, KT * G4H), F16)     # W_hh0T blocks
    w1i_d = din("w1i", (128, KT * G4H), F16)   # W_ih1T blocks (input h0)
    w1h_d = din("w1h", (128, KT * G4H), F16)   # W_hh1T blocks (input h1)
    dgt_d = din("dgt", (128, JT * 128), F16)   # row-tiled daug blocks
    b1v_d = din("b1v", (128, JT))              # b1 per j-tile
    wcat_d = din("wcat", (128, KT * O_CAT), F16)  # [W_sym; W_pos].T blocks
    bcat_d = din("bcat", (128, NB * O_CAT))    # head bias per partition
    gamw_d = din("gamw", (128, KT))
    betw_d = din("betw", (128, KT))
    hc0_d = din("hc0", (128, 4 * G4H))         # h0,c0,h1,c1 wide
    tok_d = din("tok", (T, 4, B), F16)         # [ones; tok0; tok1; tok2]
    gum_d = din("gum", (T, 128, NB * O_CAT))   # gumbel, sampling layout
    out_d = nc.dram_tensor("out", [T, 128, 2 * NB], F32,
                           kind="ExternalOutput").ap()

    # DRAM internals
    hist = nc.dram_tensor("h1_hist", [T, 128, G4H], F16).ap()
    NCH = 2 * (T // P)    # one stats buffer pair per 16-step chunk
    ccin = [nc.dram_tensor(f"cc_in{x}", [2, 128, U * KT], F32).ap()
            for x in range(NCH)]
    ccout = [nc.dram_tensor(f"cc_out{x}", [2, 128, U * KT], F32,
                            addr_space="Shared").ap() for x in range(NCH)]

    with tile.TileContext(nc) as tc:
        ctx = contextlib.ExitStack()
        with ctx:
            pc = ctx.enter_context(tc.tile_pool(name="const", bufs=1))
            pst = ctx.enter_context(tc.tile_pool(name="state", bufs=1))

            # ---------- load weights (already fp16 from host) ----------
            w0 = pc.tile([128, KT * G4H], F16)
            nc.sync.dma_start(w0[:], w0_d[:])
            w1i = pc.tile([128, KT * G4H], F16)
            nc.sync.dma_start(w1i[:], w1i_d[:])
            w1h = pc.tile([128, KT * G4H], F16)
            nc.sync.dma_start(w1h[:], w1h_d[:])
            dgt = pc.tile([128, JT * 128], F16)
            nc.sync.dma_start(dgt[:], dgt_d[:])
            wcat = pc.tile([128, KT * O_CAT], F16)
            nc.sync.dma_start(wcat[:], wcat_d[:])
            b1v = pc.tile([128, JT], F32)
            nc.sync.dma_start(b1v[:], b1v_d[:])
            bcat = pc.tile([128, NB * O_CAT], F32)
            nc.sync.dma_start(bcat[:], bcat_d[:])
            gamw = pc.tile([128, KT], F32)
            nc.sync.dma_start(gamw[:], gamw_d[:])
            betw = pc.tile([128, KT], F32)
            nc.sync.dma_start(betw[:], betw_d[:])
            epsb = pc.tile([128, 1], F32)
            nc.gpsimd.memset(epsb[:], BN_EPS)

            # ---------- states: h double-buffered, c in-place ----------
            hbuf = [[pst.tile([128, G4H], F16, name=f"h{l}_{s}")
                     for s in range(2)] for l in range(2)]
            c0 = pst.tile([128, G4H], F32, name="c0")
            c1 = pst.tile([128, G4H], F32, name="c1")
            with tc.tile_pool(name="istage", bufs=1) as pi:
                ist = pi.tile([128, 4 * G4H], F32)
                nc.sync.dma_start(ist[:], hc0_d[:])
                nc.vector.tensor_copy(hbuf[0][0][:], ist[:, 0 * G4H:1 * G4H])
                nc.vector.tensor_copy(c0[:], ist[:, 1 * G4H:2 * G4H])
                nc.vector.tensor_copy(hbuf[1][0][:], ist[:, 2 * G4H:3 * G4H])
                nc.vector.tensor_copy(c1[:], ist[:, 3 * G4H:4 * G4H])

            pa = ctx.enter_context(tc.tile_pool(name="workA", bufs=1))
            ppa = ctx.enter_context(
                tc.tile_pool(name="psumA", bufs=7, space="PSUM"))
            pb = ctx.enter_context(tc.tile_pool(name="workB", bufs=1))
            ppb = ctx.enter_context(
                tc.tile_pool(name="psumB", bufs=2, space="PSUM"))
            psc = ctx.enter_context(tc.tile_pool(name="scsh", bufs=2))
            pbn = ctx.enter_context(tc.tile_pool(name="bn", bufs=2))

            # =================== phase A single step ===================
            def step_A(t, u, statsX):
                par = u % 2
                ul = u % U   # stats slot within the chunk
                h0o, h0n = hbuf[0][par], hbuf[0][1 - par]
                h1o, h1n = hbuf[1][par], hbuf[1][1 - par]
                tokr = pa.tile([128, B], F16, tag="tokr", bufs=3)
                for sb in range(4):
                    nc.sync.dma_start(tokr[32 * sb:32 * sb + 4, :], tok_d[t])

                # ----- layer 0: gate matmuls (jb-major) + packed daug -----
                gates = [pa.tile([128, G4H], F16, tag=f"gate{q}", bufs=2,
                                 name=f"g{q}") for q in range(4)]
                pss = {}
                for jb in range(NB):
                    for q in range(4):
                        j = q * NB + jb
                        ps = ppa.tile([128, 512], F32, tag="ps",
                                      name=f"ps0_{jb}_{q}")
                        pss[q] = ps
                        for k in range(KT):
                            nc.tensor.matmul(
                                ps[:],
                                w0[:, k * G4H + j * 128:k * G4H + (j + 1) * 128],
                                h0o[:, k * 512:(k + 1) * 512],
                                start=(k == 0), stop=False)
                    # 4 rank-4 input matmuls packed into the 4 row strips
                    for q in range(4):
                        j = q * NB + jb
                        nc.tensor.matmul(
                            pss[q][:],
                            dgt[32 * q:32 * q + 4, j * 128:(j + 1) * 128],
                            tokr[32 * q:32 * q + 4, :],
                            start=False, stop=True, tile_position=(32 * q, 0))
                    blk = slice(jb * 512, (jb + 1) * 512)
                    for q in range(4):
                        func = AF.Tanh if q == 2 else AF.Sigmoid
                        nc.scalar.activation(gates[q][:, blk], pss[q][:], func)
                    # cell update for this block (h0 double-buffered)
                    t1 = pa.tile([128, 512], F32, tag="t1", bufs=2,
                                 name=f"t1_0_{jb}")
                    nc.vector.tensor_mul(t1[:], gates[1][:, blk], c0[:, blk])
                    t2 = pa.tile([128, 512], F16, tag="t2", bufs=2,
                                 name=f"t2_0_{jb}")
                    nc.vector.tensor_mul(t2[:], gates[0][:, blk], gates[2][:, blk])
                    nc.vector.tensor_add(c0[:, blk], t1[:], t2[:])
                    tnc = pa.tile([128, 512], F16, tag="tnc", bufs=2,
                                  name=f"tnc_0_{jb}")
                    nc.scalar.activation(tnc[:], c0[:, blk], AF.Tanh)
                    nc.vector.tensor_mul(h0n[:, blk], gates[3][:, blk], tnc[:])

                # ----- layer 1 (jb-major; h1 part staggered one tile ahead of
                # the h0-new part so the first h0 reads never outrun the cell) -----
                gates1 = [pa.tile([128, G4H], F16, tag=f"gate{q}", bufs=2,
                                  name=f"g1{q}") for q in range(4)]
                ps1 = {}

                def l1_hpart(jb, q):
                    j = q * NB + jb
                    ps = ppa.tile([128, 512], F32, tag="ps",
                                  name=f"ps1_{jb}_{q}")
                    ps1[(jb, q)] = ps
                    for k in range(KT):
                        nc.tensor.matmul(
                            ps[:],
                            w1h[:, k * G4H + j * 128:k * G4H + (j + 1) * 128],
                            h1o[:, k * 512:(k + 1) * 512],
                            start=(k == 0), stop=False)

                def l1_ipart(jb, q):
                    j = q * NB + jb
                    ps = ps1.pop((jb, q))
                    for k in range(KT):
                        nc.tensor.matmul(
                            ps[:],
                            w1i[:, k * G4H + j * 128:k * G4H + (j + 1) * 128],
                            h0n[:, k * 512:(k + 1) * 512],
                            start=False, stop=(k == KT - 1))
                    blk = slice(jb * 512, (jb + 1) * 512)
                    func = AF.Tanh if q == 2 else AF.Sigmoid
                    nc.scalar.activation(gates1[q][:, blk], ps[:], func,
                                         bias=b1v[:, j:j + 1])

                def l1_cell(jb):
                    blk = slice(jb * 512, (jb + 1) * 512)
                    t1 = pa.tile([128, 512], F32, tag="t1", bufs=2,
                                 name=f"t1_1_{jb}")
                    nc.vector.tensor_mul(t1[:], gates1[1][:, blk], c1[:, blk])
                    t2 = pa.tile([128, 512], F16, tag="t2", bufs=2,
                                 name=f"t2_1_{jb}")
                    nc.vector.tensor_mul(t2[:], gates1[0][:, blk],
                                         gates1[2][:, blk])
                    nc.vector.tensor_add(c1[:, blk], t1[:], t2[:])
                    tnc = pa.tile([128, 512], F16, tag="tnc", bufs=2,
                                  name=f"tnc_1_{jb}")
                    nc.scalar.activation(tnc[:], c1[:, blk], AF.Tanh)
                    nc.vector.tensor_mul(h1n[:, blk], gates1[3][:, blk], tnc[:])
                    # batch stats (sum, sum of squares) for this block
                    sqs = pa.tile([128, 512], F32, tag="sqs", bufs=2,
                                  name=f"sqs_{jb}")
                    nc.scalar.activation(
                        sqs[:], h1n[:, blk], AF.Identity,
                        accum_out=statsX[:, ul * KT + jb:ul * KT + jb + 1])
                    sqs2 = pa.tile([128, 512], F32, tag="sqs2", bufs=2,
                                   name=f"sqs2_{jb}")
                    nc.scalar.activation(
                        sqs2[:], h1n[:, blk], AF.Square,
                        accum_out=statsX[:, 4 * U + ul * KT + jb:
                                         4 * U + ul * KT + jb + 1])

                seq = [(jb, q) for jb in range(NB) for q in range(4)]
                STG = 2  # h1-part tiles staged ahead of the h0-part
                for si in range(STG):
                    l1_hpart(*seq[si])
                for si in range(JT):
                    if si + STG < JT:
                        l1_hpart(*seq[si + STG])
                    l1_ipart(*seq[si])
                    if seq[si][1] == 3:
                        l1_cell(seq[si][0])
                nc.sync.dma_start(hist[t], h1n[:])

            # =================== BN scale/shift for one chunk ===================
            def bn_chunk(scale, shift, col0, ccoX):
                W = U * KT  # 64 cols
                cs = slice(col0, col0 + W)
                g1 = pbn.tile([128, W], F32, tag="g1")
                nc.sync.dma_start(g1[:], ccoX[0])
                g2 = pbn.tile([128, W], F32, tag="g2")
                nc.sync.dma_start(g2[:], ccoX[1])
                mean = pbn.tile([128, W], F32, tag="mean")
                nc.vector.tensor_scalar(mean[:], g1[:], 1.0 / B_GLOBAL,
                                        None, op0=ALU.mult)
                var = pbn.tile([128, W], F32, tag="var")
                nc.vector.tensor_scalar(var[:], g2[:], 1.0 / B_GLOBAL,
                                        None, op0=ALU.mult)
                msq = pbn.tile([128, W], F32, tag="msq")
                nc.vector.tensor_mul(msq[:], mean[:], mean[:])
                nc.vector.tensor_sub(var[:], var[:], msq[:])
                lnv = pbn.tile([128, W], F32, tag="lnv")
                nc.scalar.activation(lnv[:], var[:], AF.Ln, bias=epsb[:])
                rstd = pbn.tile([128, W], F32, tag="rstd")
                nc.scalar.activation(rstd[:], lnv[:], AF.Exp, scale=-0.5)
                gam_bc = gamw[:].unsqueeze(1).broadcast_to([128, U, KT])
                bet_bc = betw[:].unsqueeze(1).broadcast_to([128, U, KT])
                sc3 = scale[:, cs].rearrange("p (u k) -> p u k", k=KT)
                sh3 = shift[:, cs].rearrange("p (u k) -> p u k", k=KT)
                nc.vector.tensor_tensor(
                    sc3, rstd[:].rearrange("p (u k) -> p u k", k=KT),
                    gam_bc, op=ALU.mult)
                nc.vector.tensor_mul(msq[:], mean[:], scale[:, cs])
                nc.vector.tensor_tensor(
                    sh3, bet_bc,
                    msq[:].rearrange("p (u k) -> p u k", k=KT),
                    op=ALU.subtract)

            # =================== phase B single step ===================
            def step_B(vt, ub, scale, shift, s_acc, out_acc):
                h1t = pb.tile([128, G4H], F16, tag="h1t", bufs=3)
                nc.sync.dma_start(h1t[:], hist[vt])
                gum = pb.tile([128, NB * O_CAT], F32, tag="gum", bufs=3)
                nc.sync.dma_start(gum[:], gum_d[vt])

                ysq = pb.tile([128, G4H], F16, tag="ysq", bufs=2)
                for k in range(KT):
                    c = ub * KT + k
                    nc.scalar.activation(
                        ysq[:, k * 512:(k + 1) * 512],
                        h1t[:, k * 512:(k + 1) * 512], AF.Square,
                        bias=shift[:, c:c + 1], scale=scale[:, c:c + 1])
                gaus = pb.tile([128, G4H], F16, tag="gaus", bufs=2)
                nc.scalar.activation(gaus[:], ysq[:], AF.Exp, scale=-1.0)

                ps = ppb.tile([128, NB * O_CAT], F32, tag="psb", bufs=1)
                for bb in range(NB):
                    sl = slice(bb * O_CAT, (bb + 1) * O_CAT)
                    for k in range(KT):
                        nc.tensor.matmul(
                            ps[:, sl],
                            gaus[:, k * 512 + bb * 128:k * 512 + (bb + 1) * 128],
                            wcat[:, k * O_CAT:(k + 1) * O_CAT],
                            start=(k == 0), stop=(k == KT - 1))
                z = pb.tile([128, NB * O_CAT], F32, tag="z", bufs=2)
                nc.vector.tensor_add(z[:], ps[:], bcat[:])

                z3 = z[:].rearrange("p (b o) -> p b o", b=NB)
                # softmax denominators without max-subtraction (|z| small)
                ez = pb.tile([128, NB * O_CAT], F32, tag="ez", bufs=2)
                nc.scalar.activation(ez[:], z[:], AF.Exp)
                ez3 = ez[:].rearrange("p (b o) -> p b o", b=NB)
                s8 = s_acc[:, ub * 8:(ub + 1) * 8]
                nc.vector.tensor_reduce(s8[:, 0:NB], ez3[:, :, 0:O_SYM],
                                        axis=AX.X, op=ALU.add)
                nc.vector.tensor_reduce(s8[:, NB:], ez3[:, :, O_SYM:O_CAT],
                                        axis=AX.X, op=ALU.add)
                # gumbel-max: lp_sel = (z+gum)_max - gum_sel - ln(s)
                tg = pb.tile([128, NB * O_CAT], F32, tag="tg", bufs=2)
                nc.vector.tensor_add(tg[:], z[:], gum[:])
                tg3 = tg[:].rearrange("p (b o) -> p b o", b=NB)
                t8 = pb.tile([128, 2 * NB], F32, tag="t8", bufs=2)
                nc.vector.tensor_reduce(t8[:, 0:NB], tg3[:, :, 0:O_SYM],
                                        axis=AX.X, op=ALU.max)
                nc.vector.tensor_reduce(t8[:, NB:], tg3[:, :, O_SYM:O_CAT],
                                        axis=AX.X, op=ALU.max)
                mask = pb.tile([128, NB * O_CAT], F32, tag="mask", bufs=2)
                mask3 = mask[:].rearrange("p (b o) -> p b o", b=NB)
                nc.vector.tensor_tensor(
                    mask3[:, :, 0:O_SYM], tg3[:, :, 0:O_SYM],
                    t8[:, 0:NB].unsqueeze(2).broadcast_to([128, NB, O_SYM]),
                    op=ALU.is_equal)
                nc.vector.tensor_tensor(
                    mask3[:, :, O_SYM:O_CAT], tg3[:, :, O_SYM:O_CAT],
                    t8[:, NB:].unsqueeze(2).broadcast_to([128, NB, O_POS]),
                    op=ALU.is_equal)
                gsel = pb.tile([128, NB * O_CAT], F32, tag="gsel", bufs=2)
                nc.vector.tensor_mul(gsel[:], gum[:], mask[:])
                gsel3 = gsel[:].rearrange("p (b o) -> p b o", b=NB)
                g8 = pb.tile([128, 2 * NB], F32, tag="g8", bufs=2)
                nc.vector.tensor_reduce(g8[:, 0:NB], gsel3[:, :, 0:O_SYM],
                                        axis=AX.X, op=ALU.add)
                nc.vector.tensor_reduce(g8[:, NB:], gsel3[:, :, O_SYM:O_CAT],
                                        axis=AX.X, op=ALU.add)
                nc.vector.tensor_sub(out_acc[:, ub * 8:(ub + 1) * 8],
                                     t8[:], g8[:])

            def pair_B(vt0, scale, shift):
                """One pair (P steps) of phase B work; vt0 = loop-var expr."""
                s_acc = pb.tile([128, P * 8], F32, tag="s_acc", bufs=2)
                out_acc = pb.tile([128, P * 8], F32, tag="out_acc", bufs=2)
                for ub in range(P):
                    step_B(vt0 + ub, ub, scale, shift, s_acc, out_acc)
                ln_acc = pb.tile([128, P * 8], F32, tag="ln_acc", bufs=2)
                nc.scalar.activation(ln_acc[:], s_acc[:], AF.Ln)
                nc.vector.tensor_sub(out_acc[:], out_acc[:], ln_acc[:])
                return out_acc

            def emit_pair_out(out_acc, tv):
                nc.sync.dma_start(
                    out_d[bass.ts(tv, P)].transpose([1, 0, 2]),
                    out_acc[:].rearrange("p (t c) -> p t c", c=8))

            RG = [list(range(N_CORES))]

            # ========== main sweep: unrolled pairs; B rides one pair behind ==========
            scaleE = shiftE = sE_acc = oE_acc = None
            for tv in range(NIT):
                last = tv == NIT - 1
                statsA = pa.tile([128, 8 * U], F32, tag="statsA", bufs=2,
                                 name=f"stA{tv}")
                statsB = pa.tile([128, 8 * U], F32, tag="statsB", bufs=2,
                                 name=f"stB{tv}")
                do_B = mode == "full" and tv >= 1
                if do_B:
                    scale = psc.tile([128, P * KT], F32, tag="scale",
                                     name=f"sc{tv}")
                    shift = psc.tile([128, P * KT], F32, tag="shift",
                                     name=f"sh{tv}")
                    s_acc = pb.tile([128, P * 8], F32, tag="s_acc",
                                    bufs=2, name=f"sa{tv}")
                    out_acc = pb.tile([128, P * 8], F32, tag="out_acc",
                                      bufs=2, name=f"oa{tv}")
                    vb0 = (tv - 1) * P  # first step of the pair B processes

                # first two A steps lead; BN math for the previous pair
                step_A(tv * P + 0, 0, statsA)
                step_A(tv * P + 1, 1, statsA)
                if do_B:
                    bn_chunk(scale, shift, 0, ccout[2 * tv - 2])
                    bn_chunk(scale, shift, U * KT, ccout[2 * tv - 1])
                for u in range(2, P):
                    step_A(tv * P + u, u, statsA if u < U else statsB)
                    if do_B:
                        step_B(vb0 + (u - 2), u - 2, scale, shift,
                               s_acc, out_acc)
                    if u == U - 1:
                        nc.sync.dma_start(ccin[2 * tv][0], statsA[:, 0:4 * U])
                        nc.sync.dma_start(ccin[2 * tv][1],
                                          statsA[:, 4 * U:8 * U])
                        if mode != "a":
                            nc.gpsimd.collective_compute(
                                "AllReduce", ALU.add, replica_groups=RG,
                                ins=[ccin[2 * tv].opt()],
                                outs=[ccout[2 * tv].opt()])
                    if mode == "full" and last and u == U + 1:
                        # start pulling the final pair's first chunk of B
                        # work into this iteration (its stats just reduced)
                        scaleE = psc.tile([128, P * KT], F32, tag="scale",
                                          name="scE")
                        shiftE = psc.tile([128, P * KT], F32, tag="shift",
                                          name="shE")
                        sE_acc = pb.tile([128, P * 8], F32, tag="s_acc",
                                         bufs=2, name="saE")
                        oE_acc = pb.tile([128, P * 8], F32, tag="out_acc",
                                         bufs=2, name="oaE")
                        bn_chunk(scaleE, shiftE, 0, ccout[2 * tv])
                    if mode == "full" and last and u >= U + 2:
                        ub2 = u - (U + 2)
                        step_B(tv * P + ub2, ub2, scaleE, shiftE,
                               sE_acc, oE_acc)
                if do_B:
                    step_B(vb0 + (P - 2), P - 2, scale, shift, s_acc, out_acc)
                    step_B(vb0 + (P - 1), P - 1, scale, shift, s_acc, out_acc)
                nc.sync.dma_start(ccin[2 * tv + 1][0], statsB[:, 0:4 * U])
                nc.sync.dma_start(ccin[2 * tv + 1][1], statsB[:, 4 * U:8 * U])
                if mode != "a":
                    nc.gpsimd.collective_compute(
                        "AllReduce", ALU.add, replica_groups=RG,
                        ins=[ccin[2 * tv + 1].opt()],
                        outs=[ccout[2 * tv + 1].opt()])
                if do_B:
                    ln_acc = pb.tile([128, P * 8], F32, tag="ln_acc", bufs=2,
                                     name=f"ln{tv}")
                    nc.scalar.activation(ln_acc[:], s_acc[:], AF.Ln)
                    nc.vector.tensor_sub(out_acc[:], out_acc[:], ln_acc[:])
                    emit_pair_out(out_acc, tv - 1)

            # ============ epilogue: finish the final pair of phase B ============
            if mode == "full":
                bn_chunk(scaleE, shiftE, U * KT, ccout[NCH - 1])
                for ub in range(P - U - 2, P):
                    step_B(T - P + ub, ub, scaleE, shiftE, sE_acc, oE_acc)
                ln_acc = pb.tile([128, P * 8], F32, tag="ln_acc", bufs=2,
                                 name="lnE")
                nc.scalar.activation(ln_acc[:], sE_acc[:], AF.Ln)
                nc.vector.tensor_sub(oE_acc[:], oE_acc[:], ln_acc[:])
                nc.sync.dma_start(
                    out_d[T - P:T].transpose([1, 0, 2]),
                    oE_acc[:].rearrange("p (t c) -> p t c", c=8))

    nc.compile()
    return nc


def prep_inputs(emb, W_in, b_in, W_ih0, W_hh0, b0, W_ih1, W_hh1, b1,
                gamma, beta, W_sym, b_sym, W_pos, b_pos,
                h_init, c_init, tokens, gumbel_sym, gumbel_pos, T):
    """Host-side preprocessing -> per-core input maps."""
    f64 = np.float64

    def wide(mat_t, dt=np.float16):  # [H, N] -> [128, KT*N]
        Hh, N = mat_t.shape
        return np.ascontiguousarray(
            mat_t.reshape(Hh // 128, 128, N).transpose(1, 0, 2).reshape(128, -1)
        ).astype(dt)

    w0_h = wide(W_hh0.T)
    w1i_h = wide(W_ih1.T)
    w1h_h = wide(W_hh1.T)

    Wc = W_ih0.astype(f64) @ W_in.astype(f64)            # [2048, 24]
    embd = emb.astype(f64)
    base = np.tile(embd[0], 3)                           # [24]
    delta = embd[1] - embd[0]                            # [8]
    c0v = Wc @ base + b0.astype(f64) + b_in.astype(f64) @ W_ih0.T.astype(f64)
    dvecs = [Wc[:, 8 * j:8 * (j + 1)] @ delta for j in range(3)]
    daug_h = np.stack([c0v] + dvecs).astype(np.float32)  # [4, 2048]
    # row-tiled daug: block for j-tile j=q*NB+jb lives at partitions 32q..32q+4
    dgt_h = np.zeros((128, JT * 128), np.float16)
    for j in range(JT):
        q = j // NB
        dgt_h[32 * q:32 * q + 4, j * 128:(j + 1) * 128] = \
            daug_h[:, j * 128:(j + 1) * 128]

    b1v_h = np.ascontiguousarray(b1.reshape(JT, 128).T).astype(np.float32)
    Wcat = np.concatenate([W_sym, W_pos], axis=0)        # [67, 512]
    wcat_h = wide(Wcat.T)
    bcat_h = np.tile(np.concatenate([b_sym, b_pos])[None, :],
                     (128, NB)).astype(np.float32)
    gamw_h = np.ascontiguousarray(gamma.reshape(KT, 128).T).astype(np.float32)
    betw_h = np.ascontiguousarray(beta.reshape(KT, 128).T).astype(np.float32)

    in_maps = []
    for c in range(N_CORES):
        bs = slice(c * B, (c + 1) * B)
        hc = np.concatenate([
            wide(h_init[0, bs].T, np.float32), wide(c_init[0, bs].T, np.float32),
            wide(h_init[1, bs].T, np.float32), wide(c_init[1, bs].T, np.float32)],
            axis=1)
        tok_h = np.empty((T, 4, B), np.float16)
        tok_h[:, 0, :] = 1.0
        tok_h[:, 1:4, :] = tokens[:, bs, :].transpose(0, 2, 1).astype(np.float16)
        gcat = np.concatenate(
            [gumbel_sym[:, bs, :], gumbel_pos[:, bs, :]], axis=2
        ).astype(np.float32)
        gum_h = np.ascontiguousarray(
            gcat.reshape(T, NB, 128, O_CAT).transpose(0, 2, 1, 3)
            .reshape(T, 128, NB * O_CAT))
        in_maps.append({
            "w0": w0_h, "w1i": w1i_h, "w1h": w1h_h, "dgt": dgt_h,
            "b1v": b1v_h, "wcat": wcat_h, "bcat": bcat_h,
            "gamw": gamw_h, "betw": betw_h,
            "hc0": np.ascontiguousarray(hc),
            "tok": tok_h, "gum": gum_h,
        })
    return in_maps


_NC_CACHE = {}


def run(inputs: dict, T: int, trace: bool = False):
    if T not in _NC_CACHE:
        _NC_CACHE[T] = build(T)
    nc = _NC_CACHE[T]
    in_maps = prep_inputs(T=T, **inputs)
    try:
        res = run_bass_kernel_spmd(nc, in_maps, core_ids=list(range(N_CORES)),
                                   trace=trace)
    except Exception:
        # a previous crash can leave the device wedged; reset and retry once
        try:
            import ctypes
            ctypes.CDLL("/opt/axon/libaxon_pjrt.so").axon_reset()
        except Exception:
            pass
        res = run_bass_kernel_spmd(nc, in_maps, core_ids=list(range(N_CORES)),
                                   trace=trace)
    # per-core staging [T, 128, 2*NB] -> [2, T, 512]
    outs = [r["out"].reshape(T, 128, 2, NB).transpose(2, 0, 3, 1)
            .reshape(2, T, B) for r in res.results]
    out = np.concatenate(outs, axis=2)
    return out, res


def kernel(**inputs) -> np.ndarray:
    inputs = {k: np.asarray(v) for k, v in inputs.items()}
    T = inputs["tokens"].shape[0]
    out, _ = run(inputs, T)
    return out.astype(np.float32)
